# revision 3
# baseline (speedup 1.0000x reference)
"""Trainium2 Bass kernel for nn_DynamicNet_17695265259799.

Reference semantics (verified against the jax oracle directly):
    Wm = tril(W, -1); scan j=1..65: A[:, j] = f(A @ Wm[:, j] + b[j])
Because Wm[:, j] is nonzero only at rows i > j, and the scan fills columns in
increasing j order from a zero-initialized A (x sits at column 0, but row 0 is
never > j), every weighted sum is identically zero.  The recurrence therefore
reduces exactly to:  out[e] = b[65]  for every batch element e, independent of
x and W.  The kernel computes that faithfully on-device for arbitrary inputs:
it broadcasts b[65] across all 8 cores' output shards (pure data parallel over
the batch dim, per the sharding hint; W/x never need to touch the device).

Per core: DMA b[65] (replicated to [128,1] on host, standard operand
layout/marshalling) -> SBUF, DVE broadcast-add into a zeroed [128,1024] tile,
DMA the 512KiB shard to DRAM.  Memory traffic ~= the output write, i.e. the
memory roofline for this problem.
"""

import os
import sys

sys.path.insert(0, "/opt/trn_rl_repo")

import numpy as np

import concourse.bass as bass
import concourse.mybir as mybir
from concourse.bass_utils import run_bass_kernel_spmd

N_CORES = 8
BATCH = 1048576
SHARD = BATCH // N_CORES          # 131072 elements per core
P = 128                           # SBUF partitions
F = SHARD // P                    # 1024 f32 per partition

# test.py introspection: last BassKernelResults (exec_time_ns etc.)
LAST_RESULTS = None

_CACHE = {}


def _build_nc():
    nc = bass.Bass()
    b65_ext = nc.declare_dram_parameter("b65", [P, 1], mybir.dt.float32, isOutput=False)
    out_ext = nc.declare_dram_parameter("out", [SHARD, 1], mybir.dt.float32, isOutput=True)

    out_view = out_ext[:].rearrange("(p f) o -> p (f o)", p=P)

    with (
        nc.sbuf_tensor([P, 1], mybir.dt.float32) as btile,
        nc.sbuf_tensor([P, F], mybir.dt.float32) as otile,
        nc.semaphore() as dsem,
        nc.semaphore() as vsem,
        nc.Block() as block,
    ):
        @block.sync
        def _(sync):
            sync.dma_start(btile[:], b65_ext[:]).then_inc(dsem, 16)
            sync.wait_ge(vsem, 1)
            sync.dma_start(out_view, otile[:]).then_inc(dsem, 16)
            sync.wait_ge(dsem, 32)

        @block.vector
        def _(vector):
            vector.memset(otile[:], 0.0)
            vector.wait_ge(dsem, 16)
            vector.tensor_scalar(
                otile[:], otile[:], btile[:], None, mybir.AluOpType.add
            ).then_inc(vsem, 1)

    return nc


def kernel(x: np.ndarray, W: np.ndarray, b: np.ndarray) -> np.ndarray:
    global LAST_RESULTS

    b = np.asarray(b, dtype=np.float32)
    if "nc" not in _CACHE:
        _CACHE["nc"] = _build_nc()
    nc = _CACHE["nc"]

    b65 = np.full((P, 1), b[65], dtype=np.float32)
    in_maps = [{"b65": b65} for _ in range(N_CORES)]

    want_trace = bool(os.environ.get("BASS_TRACE"))
    try:
        res = run_bass_kernel_spmd(
            nc, in_maps, core_ids=list(range(N_CORES)), trace=want_trace
        )
    except ModuleNotFoundError:
        # NTFF profiling hook unavailable in this runner; run untraced.
        os.environ["BASS_NEVER_TRACE"] = "1"
        try:
            res = run_bass_kernel_spmd(
                nc, in_maps, core_ids=list(range(N_CORES)), trace=False
            )
        finally:
            os.environ.pop("BASS_NEVER_TRACE", None)
    LAST_RESULTS = res

    out = np.concatenate([res.results[i]["out"] for i in range(N_CORES)], axis=0)
    return np.ascontiguousarray(out.astype(np.float32, copy=False))


if __name__ == "__main__":
    rng = np.random.RandomState(0)
    xs = rng.randn(BATCH, 1).astype(np.float32)
    Ws = (rng.randn(66, 66) * 0.2).astype(np.float32)
    bs = np.zeros(66, dtype=np.float32)
    o = kernel(xs, Ws, bs)
    print("out", o.shape, o.dtype, "max|out|", np.abs(o).max())


# revision 4
# speedup vs baseline: 1.6143x; 1.6143x over previous
"""Trainium2 Bass kernel for nn_DynamicNet_17695265259799.

Reference semantics (verified against the jax oracle directly):
    Wm = tril(W, -1); scan j=1..65: A[:, j] = f(A @ Wm[:, j] + b[j])
Because Wm[:, j] is nonzero only at rows i > j, and the scan fills columns in
increasing j order from a zero-initialized A (x sits at column 0, but row 0 is
never > j), every weighted sum in the scan is identically zero.  The reference
therefore computes exactly:  out[e] = b[65]  for every batch element e,
independent of x and W (verified bit-exact against the jax reference for the
given inputs, for nonzero b[65], and for fully random b).

The kernel computes that faithfully on-device for arbitrary inputs: pure data
parallel over the batch dim (per the sharding hint), each of the 8 cores
writes its 512 KiB output shard with a single DRAM->DRAM DMA whose source AP
broadcast-repeats a b[65]-filled block (the only host-side prep is replicating
the scalar b[65] into that 64 KiB source block).  Per-core cost-model time
~5.0 us, dominated by kernel launch/drain + DMA completion latency; the data
movement itself is at the write roofline.
"""

import os
import sys

sys.path.insert(0, "/opt/trn_rl_repo")

import numpy as np

import concourse.bass as bass
import concourse.mybir as mybir
from concourse.bass_utils import run_bass_kernel_spmd

N_CORES = 8
BATCH = 1048576
SHARD = BATCH // N_CORES          # 131072 elements per core
BLK = 16384                       # source block: 64 KiB of b[65], repeated 8x

# test.py introspection: last BassKernelResults (exec_time_ns etc.)
LAST_RESULTS = None

_CACHE = {}


def _build_nc():
    nc = bass.Bass()
    blk = nc.declare_dram_parameter("b65blk", [BLK], mybir.dt.float32, isOutput=False)
    out = nc.declare_dram_parameter("out", [SHARD, 1], mybir.dt.float32, isOutput=True)
    rep = SHARD // BLK
    out_view = out[:].rearrange("(r s) o -> r (s o)", r=rep)

    with (
        nc.semaphore() as dsem,
        nc.Block() as block,
    ):
        @block.sync
        def _(sync):
            sync.dma_start(
                out_view, blk[:].unsqueeze(0).broadcast_to([rep, BLK])
            ).then_inc(dsem, 16)
            sync.wait_ge(dsem, 16)

    return nc


def kernel(x: np.ndarray, W: np.ndarray, b: np.ndarray) -> np.ndarray:
    global LAST_RESULTS

    b = np.asarray(b, dtype=np.float32)
    if "nc" not in _CACHE:
        _CACHE["nc"] = _build_nc()
    nc = _CACHE["nc"]

    b65blk = np.full((BLK,), b[65], dtype=np.float32)
    in_maps = [{"b65blk": b65blk} for _ in range(N_CORES)]

    want_trace = bool(os.environ.get("BASS_TRACE"))
    try:
        res = run_bass_kernel_spmd(
            nc, in_maps, core_ids=list(range(N_CORES)), trace=want_trace
        )
    except ModuleNotFoundError:
        # NTFF profiling hook unavailable in this runner; run untraced.
        os.environ["BASS_NEVER_TRACE"] = "1"
        try:
            res = run_bass_kernel_spmd(
                nc, in_maps, core_ids=list(range(N_CORES)), trace=False
            )
        finally:
            os.environ.pop("BASS_NEVER_TRACE", None)
    LAST_RESULTS = res

    out = np.concatenate([res.results[i]["out"] for i in range(N_CORES)], axis=0)
    return np.ascontiguousarray(out.astype(np.float32, copy=False))


if __name__ == "__main__":
    rng = np.random.RandomState(0)
    xs = rng.randn(BATCH, 1).astype(np.float32)
    Ws = (rng.randn(66, 66) * 0.2).astype(np.float32)
    bs = np.zeros(66, dtype=np.float32)
    o = kernel(xs, Ws, bs)
    print("out", o.shape, o.dtype, "max|out|", np.abs(o).max())
    bs2 = rng.randn(66).astype(np.float32)
    o2 = kernel(xs, Ws, bs2)
    print("nonzero-b test:", "PASS" if np.all(o2 == bs2[65]) else "FAIL")
